# revision 3
# baseline (speedup 1.0000x reference)
"""Trainium2 Bass kernel for nn_Discriminator (LSTM + conv branch + MLP head).

Data-parallel over 8 NeuronCores: batch 512 -> 64 per core, weights replicated.

Per-core program layout (feature-on-partition "transposed" orientation for
everything except the LSTM cell state):
  - x is pre-transposed on host to xT (F+1 x T*Bc, t-major) so it serves both
    the LSTM input projection (as lhsT) and the conv branch (as rhs).
  - LSTM: z_t = [x_t,1] @ Wx_aug + h @ Wh accumulated in PSUM (Bc x 4H),
    gate columns permuted to [i f o | g] so one sigmoid op covers i,f,o.
    h is transposed each step via the PE (needed as lhsT next step).
  - Conv branch: convT = lrelu(Wc^T @ xT) in (CF x T*Bc) layout; the big
    dense (T*CF x H) accumulates out2T += Wd_chunk^T @ convT_t over t.
  - All BatchNorms + the flatten-dense bias are folded into W1/b1 on host.
  - MLP runs in transposed orientation (features on partition) so biases and
    leaky-relu fuse into single ACT ops per tile.
"""

import numpy as np

import concourse.bass as bass
import concourse.mybir as mybir
import concourse.tile as tile
from concourse import bacc, bass_utils
from concourse.masks import make_identity

F32 = mybir.dt.float32
F32R = mybir.dt.float32r
BF16 = mybir.dt.bfloat16

B, T, F, H, CF = 512, 256, 64, 256, 128
N_CORES = 8
BC = B // N_CORES  # 64
EPS = 1e-3
NT = T * BC  # 16384 columns of xT / convT

_CACHE = {}


def _build_nc():
    nc = bacc.Bacc("TRN2", target_bir_lowering=False, debug=False,
                   num_devices=N_CORES)

    d = {}
    d["xT"] = nc.dram_tensor("xT", [128, NT], F32R, kind="ExternalInput").ap()
    d["wxa"] = nc.dram_tensor("wxa", [128, 4 * H], F32R, kind="ExternalInput").ap()
    d["whp"] = nc.dram_tensor("whp", [128, 2 * 4 * H], F32R, kind="ExternalInput").ap()
    d["wcp"] = nc.dram_tensor("wcp", [128, CF], F32R, kind="ExternalInput").ap()
    d["wdp"] = nc.dram_tensor("wdp", [128, T * 2 * 128], BF16, kind="ExternalInput").ap()
    d["bdp"] = nc.dram_tensor("bdp", [128, 2], F32, kind="ExternalInput").ap()
    d["w1p"] = nc.dram_tensor("w1p", [128, 4 * 8 * 128], BF16, kind="ExternalInput").ap()
    d["b1p"] = nc.dram_tensor("b1p", [128, 8], F32, kind="ExternalInput").ap()
    d["w2p"] = nc.dram_tensor("w2p", [128, 8 * 8 * 128], BF16, kind="ExternalInput").ap()
    d["b2p"] = nc.dram_tensor("b2p", [128, 8], F32, kind="ExternalInput").ap()
    d["w3p"] = nc.dram_tensor("w3p", [128, 8], BF16, kind="ExternalInput").ap()
    d["b3p"] = nc.dram_tensor("b3p", [1, 1], F32, kind="ExternalInput").ap()
    out_d = nc.dram_tensor("out", [BC, 1], F32, kind="ExternalOutput").ap()

    PRELU = mybir.ActivationFunctionType.Prelu
    SIGM = mybir.ActivationFunctionType.Sigmoid
    TANH = mybir.ActivationFunctionType.Tanh
    MUL = mybir.AluOpType.mult
    ADD = mybir.AluOpType.add

    with tile.TileContext(nc) as tc:
        with (
            tc.tile_pool(name="const", bufs=1) as const,
            tc.tile_pool(name="wds_p", bufs=2) as wds_p,
            tc.tile_pool(name="gates", bufs=2) as gates,
            tc.tile_pool(name="state", bufs=2) as state,
            tc.tile_pool(name="tmp", bufs=3) as tmp,
            tc.tile_pool(name="ps_z", bufs=2, space="PSUM") as ps_z,
            tc.tile_pool(name="ps_tr", bufs=2, space="PSUM") as ps_tr,
            tc.tile_pool(name="ps_o2", bufs=1, space="PSUM") as ps_o2,
            tc.tile_pool(name="ps_cv", bufs=1, space="PSUM") as ps_cv,
        ):
            ident = const.tile([128, 128], F32)
            make_identity(nc, ident)

            xT = const.tile([128, NT], F32R)
            for i in range(8):
                sl = slice(i * (NT // 8), (i + 1) * (NT // 8))
                nc.sync.dma_start(out=xT[:, sl], in_=d["xT"][:, sl])

            wxa = const.tile([128, 4 * H], F32R)
            nc.sync.dma_start(out=wxa, in_=d["wxa"])
            whp = const.tile([128, 2 * 4 * H], F32R)
            nc.sync.dma_start(out=whp, in_=d["whp"])
            wcp = const.tile([128, CF], F32R)
            nc.sync.dma_start(out=wcp, in_=d["wcp"])
            bdp = const.tile([128, 2], F32)
            nc.sync.dma_start(out=bdp, in_=d["bdp"])
            w1p = const.tile([128, 4 * 8 * 128], BF16)
            nc.sync.dma_start(out=w1p, in_=d["w1p"])
            b1p = const.tile([128, 8], F32)
            nc.sync.dma_start(out=b1p, in_=d["b1p"])
            w2p = const.tile([128, 8 * 8 * 128], BF16)
            nc.sync.dma_start(out=w2p, in_=d["w2p"])
            b2p = const.tile([128, 8], F32)
            nc.sync.dma_start(out=b2p, in_=d["b2p"])
            w3p = const.tile([128, 8], BF16)
            nc.sync.dma_start(out=w3p, in_=d["w3p"])
            b3p = const.tile([1, 1], F32)
            nc.sync.dma_start(out=b3p, in_=d["b3p"])

            convT = const.tile([128, NT], BF16)
            out2T = ps_o2.tile([128, 128], F32)  # [:, 0:64]=feat 0-127, [:, 64:128]=feat 128-255

            hT = None   # (128, 128) f32r: [:, 0:64] = h^T rows 0-127
            c_prev = None

            for t in range(T):
                tb = slice(t * BC, (t + 1) * BC)

                # ---- conv branch chunk (every 8 steps) ----
                if t % 8 == 0:
                    ci = t // 8
                    cs = slice(ci * 512, (ci + 1) * 512)
                    pcv = ps_cv.tile([128, 512], F32, tag="cv")
                    nc.tensor.matmul(pcv, wcp, xT[:, cs], start=True, stop=True)
                    nc.scalar.activation(convT[:, cs], pcv, PRELU, alpha=0.2)
                    # Wd stream for the next 8 timesteps
                    wds = wds_p.tile([128, 8 * 2 * 128], BF16, tag="wds")
                    nc.sync.dma_start(
                        out=wds, in_=d["wdp"][:, t * 256:(t + 8) * 256]
                    )

                # ---- LSTM z matmuls ----
                pz = ps_z.tile([BC, 4 * H], F32, tag="z")
                for n in range(2):
                    ns = slice(n * 512, (n + 1) * 512)
                    nc.tensor.matmul(pz[:, ns], xT[:, tb], wxa[:, ns],
                                     start=True, stop=(t == 0))
                if t > 0:
                    for k in range(2):
                        for n in range(2):
                            ns = slice(n * 512, (n + 1) * 512)
                            nc.tensor.matmul(
                                pz[:, ns],
                                hT[:, k * 64:(k + 1) * 64],
                                whp[:, k * 1024 + n * 512:k * 1024 + (n + 1) * 512],
                                start=False, stop=(k == 1))

                # ---- gates: cols [i f o | g] ----
                sig = gates.tile([BC, 768], F32, tag="sig")
                nc.scalar.activation(sig, pz[:, 0:768], SIGM)
                tg = gates.tile([BC, H], F32, tag="tg")
                nc.scalar.activation(tg, pz[:, 768:1024], TANH)

                # ---- c update ----
                c_new = state.tile([BC, H], F32, tag="c")
                if t == 0:
                    nc.vector.tensor_tensor(out=c_new, in0=sig[:, 0:256], in1=tg, op=MUL)
                else:
                    q1 = tmp.tile([BC, H], F32, tag="q1")
                    nc.vector.tensor_tensor(out=q1, in0=sig[:, 256:512], in1=c_prev, op=MUL)
                    q2 = tmp.tile([BC, H], F32, tag="q2")
                    nc.vector.tensor_tensor(out=q2, in0=sig[:, 0:256], in1=tg, op=MUL)
                    nc.vector.tensor_tensor(out=c_new, in0=q1, in1=q2, op=ADD)
                c_prev = c_new

                # ---- h = sig(o) * tanh(c), then transpose for next step ----
                tc_t = tmp.tile([BC, H], F32, tag="tc")
                nc.scalar.activation(tc_t, c_new, TANH)
                h_nat = tmp.tile([BC, H], F32, tag="h")
                nc.vector.tensor_tensor(out=h_nat, in0=sig[:, 512:768], in1=tc_t, op=MUL)
                hT = state.tile([128, 128], F32R, tag="hT")
                for k in range(2):
                    ptr = ps_tr.tile([128, BC], F32, tag="tr")
                    nc.tensor.transpose(ptr, h_nat[:, k * 128:(k + 1) * 128],
                                        ident[0:BC, 0:BC])
                    nc.vector.tensor_copy(hT[:, k * 64:(k + 1) * 64], ptr)

                # ---- dense (Wd) accumulation for this timestep ----
                # NOTE: start=True clears has_written for the whole PSUM bank,
                # so only the very first matmul into this bank may set it; the
                # m=1 group starts with cleared bits -> overwrite, then accums.
                wds_off = (t % 8) * 256
                for m in range(2):
                    nc.tensor.matmul(
                        out2T[:, m * 64:(m + 1) * 64],
                        wds[:, wds_off + m * 128:wds_off + (m + 1) * 128],
                        convT[:, tb],
                        start=(t == 0 and m == 0), stop=(t == T - 1),
                        skip_group_check=True)

            # ---- u_raw^T tiles (bf16): [lrelu(h) ; lrelu(out2+bd)] ----
            uT = const.tile([128, 4 * 64], BF16)
            nc.scalar.activation(uT[:, 0:64], hT[:, 0:64].bitcast(F32), PRELU, alpha=0.2)
            nc.scalar.activation(uT[:, 64:128], hT[:, 64:128].bitcast(F32), PRELU, alpha=0.2)
            nc.scalar.activation(uT[:, 128:192], out2T[:, 0:64], PRELU,
                                 bias=bdp[:, 0:1], alpha=0.2)
            nc.scalar.activation(uT[:, 192:256], out2T[:, 64:128], PRELU,
                                 bias=bdp[:, 1:2], alpha=0.2)

            # ---- MLP in transposed orientation ----
            m1T = const.tile([128, 8 * 64], BF16)
            for m in range(8):
                pm = ps_cv.tile([128, 64], F32, tag="cv")
                for k in range(4):
                    nc.tensor.matmul(pm, w1p[:, (k * 8 + m) * 128:(k * 8 + m + 1) * 128],
                                     uT[:, k * 64:(k + 1) * 64],
                                     start=(k == 0), stop=(k == 3))
                nc.scalar.activation(m1T[:, m * 64:(m + 1) * 64], pm, PRELU,
                                     bias=b1p[:, m:m + 1], alpha=0.3)
            m2T = const.tile([128, 8 * 64], BF16)
            for m in range(8):
                pm = ps_cv.tile([128, 64], F32, tag="cv")
                for k in range(8):
                    nc.tensor.matmul(pm, w2p[:, (k * 8 + m) * 128:(k * 8 + m + 1) * 128],
                                     m1T[:, k * 64:(k + 1) * 64],
                                     start=(k == 0), stop=(k == 7))
                nc.scalar.activation(m2T[:, m * 64:(m + 1) * 64], pm, PRELU,
                                     bias=b2p[:, m:m + 1], alpha=0.3)
            po = ps_tr.tile([1, 64], F32, tag="tr")
            for k in range(8):
                nc.tensor.matmul(po, w3p[:, k:k + 1], m2T[:, k * 64:(k + 1) * 64],
                                 start=(k == 0), stop=(k == 7))
            oS = tmp.tile([1, 64], F32, tag="oS")
            nc.scalar.activation(oS, po, SIGM, bias=b3p)
            nc.sync.dma_start(out=out_d.rearrange("a b -> b a"), in_=oS)

    nc.compile()
    return nc


def _prep_weights(inputs):
    """Host-side packing of all weights (shared across cores)."""
    Wx = np.asarray(inputs["Wx"], np.float32)
    Wh = np.asarray(inputs["Wh"], np.float32)
    b_lstm = np.asarray(inputs["b_lstm"], np.float32)
    Wc = np.asarray(inputs["Wc"], np.float32)
    bc = np.asarray(inputs["bc"], np.float32)
    Wd = np.asarray(inputs["Wd"], np.float32)
    bd = np.asarray(inputs["bd"], np.float32)
    W1 = np.asarray(inputs["W1"], np.float32)
    b1 = np.asarray(inputs["b1"], np.float32)
    W2 = np.asarray(inputs["W2"], np.float32)
    b2 = np.asarray(inputs["b2"], np.float32)
    W3 = np.asarray(inputs["W3"], np.float32)
    b3 = np.asarray(inputs["b3"], np.float32)

    # gate column permutation i f g o -> i f o g
    perm = np.concatenate([np.arange(0, 512), np.arange(768, 1024),
                           np.arange(512, 768)])

    wxa = np.zeros((128, 4 * H), np.float32)
    wxa[0:F] = Wx[:, perm]
    wxa[F] = b_lstm[perm]

    whp_n = Wh[:, perm]
    whp = np.concatenate([whp_n[0:128], whp_n[128:256]], axis=1)  # (128, 2048)

    wcp = np.zeros((128, CF), np.float32)
    wcp[0:F] = Wc
    wcp[F] = bc

    # Wd: (T*CF, H) -> per (t, m) chunk (128cf x 128h)
    wd4 = Wd.reshape(T, CF, 2, 128)          # t, cf, m, j
    wdp = np.ascontiguousarray(
        wd4.transpose(1, 0, 2, 3).reshape(128, T * 2 * 128)
    ).astype(np.dtype("bfloat16"))
    bdp = np.ascontiguousarray(bd.reshape(2, 128).T)  # (128, 2)

    # fold BN1/BN2/BN3 into W1/b1
    a1 = inputs["bn1_g"] / np.sqrt(inputs["bn1_v"] + EPS)
    o1 = inputs["bn1_b"] - inputs["bn1_m"] * a1
    a2 = inputs["bn2_g"] / np.sqrt(inputs["bn2_v"] + EPS)
    o2 = inputs["bn2_b"] - inputs["bn2_m"] * a2
    a3 = inputs["bn3_g"] / np.sqrt(inputs["bn3_v"] + EPS)
    o3 = inputs["bn3_b"] - inputs["bn3_m"] * a3
    A = np.asarray(a3 * np.concatenate([a1, a2]), np.float32)       # (512,)
    Boff = np.asarray(a3 * np.concatenate([o1, o2]) + o3, np.float32)
    W1f = (A[:, None] * W1).astype(np.float32)
    b1f = (Boff @ W1 + b1).astype(np.float32)

    def pack_T(w, kc, mc):
        # (kc*128, mc*128) -> (128, kc*mc*128), chunk (k,m) at [(k*mc+m)*128]
        return np.ascontiguousarray(
            w.reshape(kc, 128, mc, 128).transpose(1, 0, 2, 3).reshape(128, kc * mc * 128)
        )

    w1p = pack_T(W1f, 4, 8).astype(np.dtype("bfloat16"))
    b1p = np.ascontiguousarray(b1f.reshape(8, 128).T)
    w2p = pack_T(W2, 8, 8).astype(np.dtype("bfloat16"))
    b2p = np.ascontiguousarray(b2.reshape(8, 128).T)
    w3p = np.ascontiguousarray(W3.reshape(8, 128, 1)[:, :, 0].T).astype(
        np.dtype("bfloat16"))  # (128, 8)
    b3p = b3.reshape(1, 1)

    return dict(wxa=wxa, whp=np.ascontiguousarray(whp), wcp=wcp, wdp=wdp,
                bdp=bdp, w1p=w1p, b1p=b1p, w2p=w2p, b2p=b2p, w3p=w3p, b3p=b3p)


def kernel(**inputs):
    if "nc" not in _CACHE:
        _CACHE["nc"] = _build_nc()
    nc = _CACHE["nc"]

    x = np.asarray(inputs["inputs"], np.float32)  # (B, T, F)
    w = _prep_weights(inputs)

    in_maps = []
    for c in range(N_CORES):
        xc = x[c * BC:(c + 1) * BC]               # (BC, T, F)
        xT = np.zeros((128, NT), np.float32)
        xT[0:F] = xc.transpose(2, 1, 0).reshape(F, NT)  # [f, t*BC+b]
        xT[F] = 1.0
        in_maps.append({"xT": xT, **w})

    res = bass_utils.run_bass_kernel_spmd(nc, in_maps, core_ids=list(range(N_CORES)))
    out = np.concatenate([res.results[c]["out"] for c in range(N_CORES)], axis=0)
    return out.astype(np.float32)


# revision 22
# speedup vs baseline: 4553.8307x; 4553.8307x over previous
"""Trainium2 Bass kernel for nn_Discriminator (LSTM + conv branch + MLP head).

Data-parallel over 8 NeuronCores: batch 512 -> 64 per core, weights replicated.

Per-core program layout (feature-on-partition "transposed" orientation for
everything except the LSTM cell state):
  - x is pre-transposed on host to xT (F+1 x T*Bc, t-major) so it serves both
    the LSTM input projection (as lhsT) and the conv branch (as rhs).
  - LSTM: z_t = [x_t,1] @ Wx_aug + h @ Wh accumulated in PSUM (Bc x 4H),
    gate columns permuted to [i f o | g] so one sigmoid op covers i,f,o.
    h is transposed each step via the PE (needed as lhsT next step).
  - Conv branch: convT = lrelu(Wc^T @ xT) in (CF x T*Bc) layout; the big
    dense (T*CF x H) accumulates out2T += Wd_chunk^T @ convT_t over t.
  - All BatchNorms + the flatten-dense bias are folded into W1/b1 on host.
  - MLP runs in transposed orientation (features on partition) so biases and
    leaky-relu fuse into single ACT ops per tile.
"""

import numpy as np

import concourse.bass as bass
import concourse.mybir as mybir
import concourse.tile as tile
from concourse import bacc, bass_utils
from concourse.masks import make_identity

F32 = mybir.dt.float32
F32R = mybir.dt.float32r
BF16 = mybir.dt.bfloat16

B, T, F, H, CF = 512, 256, 64, 256, 128
N_CORES = 8
BC = B // N_CORES  # 64
EPS = 1e-3
NT = T * BC  # 16384 columns of xT / convT

_CACHE = {}


def _build_nc():
    nc = bacc.Bacc("TRN2", target_bir_lowering=False, debug=False,
                   num_devices=N_CORES)

    d = {}
    d["xT"] = nc.dram_tensor("xT", [128, NT], F32R, kind="ExternalInput").ap()
    d["wxa"] = nc.dram_tensor("wxa", [128, 4 * H], F32R, kind="ExternalInput").ap()
    d["whp"] = nc.dram_tensor("whp", [128, 2 * 4 * H], F32R, kind="ExternalInput").ap()
    d["wcp"] = nc.dram_tensor("wcp", [128, CF], F32R, kind="ExternalInput").ap()
    d["wdp"] = nc.dram_tensor("wdp", [128, T * 2 * 128], BF16, kind="ExternalInput").ap()
    d["bdp"] = nc.dram_tensor("bdp", [128, 2], F32, kind="ExternalInput").ap()
    d["w1p"] = nc.dram_tensor("w1p", [128, 4 * 8 * 128], BF16, kind="ExternalInput").ap()
    d["b1p"] = nc.dram_tensor("b1p", [128, 8], F32, kind="ExternalInput").ap()
    d["w2p"] = nc.dram_tensor("w2p", [128, 8 * 8 * 128], BF16, kind="ExternalInput").ap()
    d["b2p"] = nc.dram_tensor("b2p", [128, 8], F32, kind="ExternalInput").ap()
    d["w3p"] = nc.dram_tensor("w3p", [128, 8], BF16, kind="ExternalInput").ap()
    d["b3p"] = nc.dram_tensor("b3p", [1, 1], F32, kind="ExternalInput").ap()
    out_d = nc.dram_tensor("out", [BC, 1], F32, kind="ExternalOutput").ap()

    PRELU = mybir.ActivationFunctionType.Prelu
    SIGM = mybir.ActivationFunctionType.Sigmoid
    TANH = mybir.ActivationFunctionType.Tanh
    MUL = mybir.AluOpType.mult
    ADD = mybir.AluOpType.add

    with tile.TileContext(nc) as tc:
        with (
            tc.tile_pool(name="const", bufs=1) as const,
            tc.tile_pool(name="wds_p", bufs=2) as wds_p,
            tc.tile_pool(name="gates", bufs=2) as gates,
            tc.tile_pool(name="state", bufs=2) as state,
            tc.tile_pool(name="tmp", bufs=3) as tmp,
            tc.tile_pool(name="ps_z", bufs=2, space="PSUM") as ps_z,
            tc.tile_pool(name="ps_tr", bufs=1, space="PSUM") as ps_tr,
            tc.tile_pool(name="ps_o2", bufs=1, space="PSUM") as ps_o2,
            tc.tile_pool(name="ps_cv", bufs=1, space="PSUM") as ps_cv,
        ):
            ident = const.tile([128, 128], F32)
            make_identity(nc, ident)

            # DMA order = priority: the t=0 x-matmuls need only the first
            # xT chunk + the LSTM weights; everything else streams in behind.
            xT = const.tile([128, NT], F32R)
            CH = NT // 8
            nc.sync.dma_start(out=xT[:, 0:CH], in_=d["xT"][:, 0:CH])
            wxa = const.tile([128, 4 * H], F32R)
            nc.sync.dma_start(out=wxa, in_=d["wxa"])
            whp = const.tile([128, 2 * 4 * H], F32R)
            nc.sync.dma_start(out=whp, in_=d["whp"])
            wcp = const.tile([128, CF], F32R)
            nc.sync.dma_start(out=wcp, in_=d["wcp"])
            for i in range(1, 8):
                sl = slice(i * CH, (i + 1) * CH)
                nc.sync.dma_start(out=xT[:, sl], in_=d["xT"][:, sl])
            bdp = const.tile([128, 2], F32)
            nc.sync.dma_start(out=bdp, in_=d["bdp"])
            w1p = const.tile([128, 4 * 8 * 128], BF16)
            nc.sync.dma_start(out=w1p, in_=d["w1p"])
            b1p = const.tile([128, 8], F32)
            nc.sync.dma_start(out=b1p, in_=d["b1p"])
            w2p = const.tile([128, 8 * 8 * 128], BF16)
            nc.sync.dma_start(out=w2p, in_=d["w2p"])
            b2p = const.tile([128, 8], F32)
            nc.sync.dma_start(out=b2p, in_=d["b2p"])
            w3p = const.tile([128, 8], BF16)
            nc.sync.dma_start(out=w3p, in_=d["w3p"])
            b3p = const.tile([1, 1], F32)
            nc.sync.dma_start(out=b3p, in_=d["b3p"])

            convT = const.tile([128, NT], BF16)
            out2T = ps_o2.tile([128, 128], F32)  # [:, 0:64]=feat 0-127, [:, 64:128]=feat 128-255

            hT = None   # pair of (128, 64) f32r tiles: h^T rows 0-127 / 128-255
            c_prev = None

            def x_mms(pzAB, t):
                # x-part of z for step t (independent of the recurrence)
                for n in range(2):
                    ns = slice(n * 512, (n + 1) * 512)
                    nc.tensor.matmul(pzAB[n], xT[:, t * BC:(t + 1) * BC],
                                     wxa[:, ns], start=True, stop=(t == 0))

            def conv_chunk(ci):
                # conv-branch chunk ci (8 timesteps) + Wd stream for it.
                cs = slice(ci * 512, (ci + 1) * 512)
                pcv = ps_cv.tile([128, 512], F32, tag="cv", name="pcv")
                nc.tensor.matmul(pcv, wcp, xT[:, cs], start=True, stop=True)
                nc.scalar.activation(convT[:, cs], pcv, PRELU, alpha=0.2)
                wds = wds_p.tile([128, 8 * 2 * 128], BF16, tag="wds", name="wds")
                nc.sync.dma_start(
                    out=wds, in_=d["wdp"][:, ci * 8 * 256:(ci + 1) * 8 * 256])
                return wds

            # z lives in two separate one-bank tiles so gate reads only wait
            # on their own half's matmuls (dep tracking is tile-granular).
            pz = (ps_z.tile([BC, 512], F32, tag="zA", name="pzA"),
                  ps_z.tile([BC, 512], F32, tag="zB", name="pzB"))
            x_mms(pz, 0)
            wds_cur = conv_chunk(0)

            for t in range(T):
                tb = slice(t * BC, (t + 1) * BC)

                if t % 8 == 0:
                    wds = wds_cur

                # ---- LSTM recurrent z matmuls (x-part was issued last iter) ----
                # n-outer order: gate cols 0:512 (i,f) complete after 2 MMs so
                # the sigmoid can start while cols 512:1024 (o,g) still stream.
                if t > 0:
                    for n in range(2):
                        for k in range(2):
                            nc.tensor.matmul(
                                pz[n],
                                hT[k],
                                whp[:, k * 1024 + n * 512:k * 1024 + (n + 1) * 512],
                                start=False, stop=(k == 1))

                # ---- gates: cols [i f o | g] ----
                sig = gates.tile([BC, 512], F32, tag="sig")
                nc.scalar.activation(sig, pz[0], SIGM)
                tg = gates.tile([BC, H], F32, tag="tg")
                nc.scalar.activation(tg, pz[1][:, 256:512], TANH)
                sig_o = gates.tile([BC, H], F32, tag="sig_o")
                nc.scalar.activation(sig_o, pz[1][:, 0:256], SIGM)

                # next step's x-part: issued here so the PE can run it while
                # ACT/DVE chew on this step's gates (PE executes in order).
                if t + 1 < T:
                    pz_next = (ps_z.tile([BC, 512], F32, tag="zA", name="pzA"),
                               ps_z.tile([BC, 512], F32, tag="zB", name="pzB"))
                    x_mms(pz_next, t + 1)

                # ---- c update ----
                # cT = q1^T + q2^T is summed IN PSUM by accumulating
                # transposes (q1^T written first, q2^T accumulated on top),
                # so the chain does not wait for the natural-layout add --
                # that add runs off-chain and only feeds next step's q1.
                pcT = ps_tr.tile([128, 128], F32, tag="pcT")
                c_new = state.tile([BC, H], F32, tag="c")
                if t == 0:
                    nc.vector.tensor_tensor(out=c_new, in0=sig[:, 0:256], in1=tg, op=MUL)
                    for k in range(2):
                        nc.tensor.matmul(pcT[:, k * 64:(k + 1) * 64],
                                         c_new[:, k * 128:(k + 1) * 128],
                                         ident[0:BC, 0:BC], is_transpose=True,
                                         start=(k == 0), stop=True,
                                         skip_group_check=True)
                else:
                    q1 = tmp.tile([BC, H], F32, tag="q1")
                    nc.vector.tensor_tensor(out=q1, in0=sig[:, 256:512], in1=c_prev, op=MUL)
                    q2 = tmp.tile([BC, H], F32, tag="q2")
                    nc.vector.tensor_tensor(out=q2, in0=sig[:, 0:256], in1=tg, op=MUL)
                    for k in range(2):
                        nc.tensor.matmul(pcT[:, k * 64:(k + 1) * 64],
                                         q1[:, k * 128:(k + 1) * 128],
                                         ident[0:BC, 0:BC], is_transpose=True,
                                         start=(k == 0), stop=False,
                                         skip_group_check=True)
                    for k in range(2):
                        nc.tensor.matmul(pcT[:, k * 64:(k + 1) * 64],
                                         q2[:, k * 128:(k + 1) * 128],
                                         ident[0:BC, 0:BC], is_transpose=True,
                                         start=False, stop=True,
                                         skip_group_check=True)
                    nc.vector.tensor_tensor(out=c_new, in0=q1, in1=q2, op=ADD)
                c_prev = c_new

                # ---- dense (Wd) accumulation for this timestep ----
                # NOTE: start=True clears has_written for the whole PSUM bank,
                # so only the very first matmul into this bank may set it; the
                # m=1 group starts with cleared bits -> overwrite, then accums.
                # Emitted before the transposes so the PE stream does not park
                # behind transposes that wait on the gate chain.
                wds_off = (t % 8) * 256
                for m in range(2):
                    nc.tensor.matmul(
                        out2T[:, m * 64:(m + 1) * 64],
                        wds[:, wds_off + m * 128:wds_off + (m + 1) * 128],
                        convT[:, tb],
                        start=(t == 0 and m == 0), stop=(t == T - 1),
                        skip_group_check=True)

                # ---- transposed tail: hT = (sig o)^T * tanh(c^T) ----
                # sig(o)^T: off the critical path (ready right after sig).
                # tanh/mul are split into hidden-halves in separate tiles so
                # next step's k0 matmul starts before the k1 half is ready
                # (dep tracking is tile-granular).
                poT = ps_tr.tile([128, 128], F32, tag="poT")
                for k in range(2):
                    nc.tensor.transpose(poT[:, k * 64:(k + 1) * 64],
                                        sig_o[:, k * 128:(k + 1) * 128],
                                        ident[0:BC, 0:BC])
                oT_sb = tmp.tile([128, 128], F32, tag="oT_sb")
                nc.vector.tensor_copy(oT_sb, poT)
                tcT = tmp.tile([128, 128], F32, tag="tcT")
                nc.scalar.activation(tcT, pcT, TANH)
                hT0 = state.tile([128, 64], F32R, tag="hT0")
                nc.vector.tensor_tensor(out=hT0, in0=oT_sb[:, 0:64], in1=tcT[:, 0:64], op=MUL)
                hT1 = state.tile([128, 64], F32R, tag="hT1")
                nc.vector.tensor_tensor(out=hT1, in0=oT_sb[:, 64:128], in1=tcT[:, 64:128], op=MUL)
                hT = (hT0, hT1)
                if t + 1 < T:
                    pz = pz_next

                # prefetch next conv chunk at the end of the iteration: the
                # conv matmul runs in PE idle before the next h-matmuls, and
                # its lrelu fits the ACT gap after tanh(c^T).
                if t % 8 == 0 and t + 8 < T:
                    wds_cur = conv_chunk(t // 8 + 1)

            # ---- u_raw^T tiles (bf16): [lrelu(h) ; lrelu(out2+bd)] ----
            # u3/u4 depend only on the dense branch (done early in step 255),
            # so they and the k=2,3 W1 chunks overlap the tail of the LSTM;
            # separate tiles keep their deps apart (tile-granular tracking).
            uh = const.tile([128, 2 * 64], BF16)
            uo = const.tile([128, 2 * 64], BF16)
            nc.scalar.activation(uo[:, 0:64], out2T[:, 0:64], PRELU,
                                 bias=bdp[:, 0:1], alpha=0.2)
            nc.scalar.activation(uo[:, 64:128], out2T[:, 64:128], PRELU,
                                 bias=bdp[:, 1:2], alpha=0.2)
            nc.scalar.activation(uh[:, 0:64], hT[0].bitcast(F32), PRELU, alpha=0.2)
            nc.scalar.activation(uh[:, 64:128], hT[1].bitcast(F32), PRELU, alpha=0.2)

            # ---- MLP in transposed orientation ----
            m1T = const.tile([128, 8 * 64], BF16)
            for m in range(8):
                pm = ps_z.tile([128, 64], F32, tag=("zA" if m % 2 == 0 else "zB"),
                               name="pm")
                # k=2,3 (dense-branch inputs) first: they run while the last
                # LSTM steps are still on the chain; k=0,1 need h_final.
                for j, k in enumerate((2, 3, 0, 1)):
                    u_src = uo if k >= 2 else uh
                    nc.tensor.matmul(pm, w1p[:, (k * 8 + m) * 128:(k * 8 + m + 1) * 128],
                                     u_src[:, (k % 2) * 64:(k % 2 + 1) * 64],
                                     start=(j == 0), stop=(j == 3))
                nc.scalar.activation(m1T[:, m * 64:(m + 1) * 64], pm, PRELU,
                                     bias=b1p[:, m:m + 1], alpha=0.3)
            m2T = const.tile([128, 8 * 64], BF16)
            for m in range(8):
                pm = ps_z.tile([128, 64], F32, tag=("zA" if m % 2 == 0 else "zB"),
                               name="pm")
                for k in range(8):
                    nc.tensor.matmul(pm, w2p[:, (k * 8 + m) * 128:(k * 8 + m + 1) * 128],
                                     m1T[:, k * 64:(k + 1) * 64],
                                     start=(k == 0), stop=(k == 7))
                nc.scalar.activation(m2T[:, m * 64:(m + 1) * 64], pm, PRELU,
                                     bias=b2p[:, m:m + 1], alpha=0.3)
            po = ps_tr.tile([1, 64], F32, tag="poT")
            for k in range(8):
                nc.tensor.matmul(po, w3p[:, k:k + 1], m2T[:, k * 64:(k + 1) * 64],
                                 start=(k == 0), stop=(k == 7))
            oS = tmp.tile([1, 64], F32, tag="oS")
            nc.scalar.activation(oS, po, SIGM, bias=b3p)
            nc.sync.dma_start(out=out_d.rearrange("a b -> b a"), in_=oS)

    nc.compile()
    return nc


def _prep_weights(inputs):
    """Host-side packing of all weights (shared across cores)."""
    Wx = np.asarray(inputs["Wx"], np.float32)
    Wh = np.asarray(inputs["Wh"], np.float32)
    b_lstm = np.asarray(inputs["b_lstm"], np.float32)
    Wc = np.asarray(inputs["Wc"], np.float32)
    bc = np.asarray(inputs["bc"], np.float32)
    Wd = np.asarray(inputs["Wd"], np.float32)
    bd = np.asarray(inputs["bd"], np.float32)
    W1 = np.asarray(inputs["W1"], np.float32)
    b1 = np.asarray(inputs["b1"], np.float32)
    W2 = np.asarray(inputs["W2"], np.float32)
    b2 = np.asarray(inputs["b2"], np.float32)
    W3 = np.asarray(inputs["W3"], np.float32)
    b3 = np.asarray(inputs["b3"], np.float32)

    # gate column permutation i f g o -> i f o g
    perm = np.concatenate([np.arange(0, 512), np.arange(768, 1024),
                           np.arange(512, 768)])

    wxa = np.zeros((128, 4 * H), np.float32)
    wxa[0:F] = Wx[:, perm]
    wxa[F] = b_lstm[perm]

    whp_n = Wh[:, perm]
    whp = np.concatenate([whp_n[0:128], whp_n[128:256]], axis=1)  # (128, 2048)

    wcp = np.zeros((128, CF), np.float32)
    wcp[0:F] = Wc
    wcp[F] = bc

    # Wd: (T*CF, H) -> per (t, m) chunk (128cf x 128h)
    wd4 = Wd.reshape(T, CF, 2, 128)          # t, cf, m, j
    wdp = np.ascontiguousarray(
        wd4.transpose(1, 0, 2, 3).reshape(128, T * 2 * 128)
    ).astype(np.dtype("bfloat16"))
    bdp = np.ascontiguousarray(bd.reshape(2, 128).T)  # (128, 2)

    # fold BN1/BN2/BN3 into W1/b1
    a1 = inputs["bn1_g"] / np.sqrt(inputs["bn1_v"] + EPS)
    o1 = inputs["bn1_b"] - inputs["bn1_m"] * a1
    a2 = inputs["bn2_g"] / np.sqrt(inputs["bn2_v"] + EPS)
    o2 = inputs["bn2_b"] - inputs["bn2_m"] * a2
    a3 = inputs["bn3_g"] / np.sqrt(inputs["bn3_v"] + EPS)
    o3 = inputs["bn3_b"] - inputs["bn3_m"] * a3
    A = np.asarray(a3 * np.concatenate([a1, a2]), np.float32)       # (512,)
    Boff = np.asarray(a3 * np.concatenate([o1, o2]) + o3, np.float32)
    W1f = (A[:, None] * W1).astype(np.float32)
    b1f = (Boff @ W1 + b1).astype(np.float32)

    def pack_T(w, kc, mc):
        # (kc*128, mc*128) -> (128, kc*mc*128), chunk (k,m) at [(k*mc+m)*128]
        return np.ascontiguousarray(
            w.reshape(kc, 128, mc, 128).transpose(1, 0, 2, 3).reshape(128, kc * mc * 128)
        )

    w1p = pack_T(W1f, 4, 8).astype(np.dtype("bfloat16"))
    b1p = np.ascontiguousarray(b1f.reshape(8, 128).T)
    w2p = pack_T(W2, 8, 8).astype(np.dtype("bfloat16"))
    b2p = np.ascontiguousarray(b2.reshape(8, 128).T)
    w3p = np.ascontiguousarray(W3.reshape(8, 128, 1)[:, :, 0].T).astype(
        np.dtype("bfloat16"))  # (128, 8)
    b3p = b3.reshape(1, 1)

    return dict(wxa=wxa, whp=np.ascontiguousarray(whp), wcp=wcp, wdp=wdp,
                bdp=bdp, w1p=w1p, b1p=b1p, w2p=w2p, b2p=b2p, w3p=w3p, b3p=b3p)


def kernel(**inputs):
    if "nc" not in _CACHE:
        _CACHE["nc"] = _build_nc()
    nc = _CACHE["nc"]

    x = np.asarray(inputs["inputs"], np.float32)  # (B, T, F)
    w = _prep_weights(inputs)

    in_maps = []
    for c in range(N_CORES):
        xc = x[c * BC:(c + 1) * BC]               # (BC, T, F)
        xT = np.zeros((128, NT), np.float32)
        xT[0:F] = xc.transpose(2, 1, 0).reshape(F, NT)  # [f, t*BC+b]
        xT[F] = 1.0
        in_maps.append({"xT": xT, **w})

    res = bass_utils.run_bass_kernel_spmd(nc, in_maps, core_ids=list(range(N_CORES)))
    out = np.concatenate([res.results[c]["out"] for c in range(N_CORES)], axis=0)
    return out.astype(np.float32)
